# revision 30
# baseline (speedup 1.0000x reference)
"""Trainium2 Bass kernel for the DeepSets-style segment_reduce network.

Network (per sample, B=2048, M=128 elements):
  phi: 3 -> 120 -> 100 -> 80 MLP (all ReLU), applied per element
  pooled = sum over the 128 elements                      [B, 80]
  rho:  80 -> 60 -> 60 -> 40 (ReLU, ReLU, linear)
  q:    concat(rho_out, static) 43 -> 200 -> 100 -> 3, softmax

Mapping: data-parallel over 8 NeuronCores (256 samples each). Activations are
feature-major [features, elements] so each layer is one stationary-weight
matmul sweep of 1024-column fp16 moving operands. Layer biases are folded
into the PSUM->SBUF relu copies (ACT bias operand / DVE tensor_scalar
add+max); only L3 keeps a ones-row (bias via augmented weights) because its
DVE half is a scalar_tensor_tensor that has no bias slot. L3's PSUM read
fuses relu + pairwise element-add (m vs m+64), and the remaining 64-wide
per-sample sum is a chunked DVE tensor_reduce. The rho/q/softmax tail runs
per 128-sample half, its serial chain interleaved between the next half's
matmul groups so the PE never idles long enough to drop out of the warm
2.4 GHz HAM state.
"""

import sys
import numpy as np

sys.path.insert(0, '/opt/trn_rl_repo')

B, M, D = 2048, 128, 3
N_CORES = 8
BC = B // N_CORES            # samples per core (256)
EC = BC * M                  # elements per core (32768)
HALF = EC // 2               # elements per half-sweep (16384)
SAMP_HALF = BC // 2          # samples per half (128)
PT = 1024                    # elements per PSUM tile
NPT = HALF // PT             # psum tiles per half (16)
GS = M // 2                  # pair-summed group width (64)

F1, F2, F3 = 120, 100, 80    # phi widths
R1, R2, R3 = 60, 60, 40      # rho widths
Q1, Q2, Q3 = 200, 100, 3     # q widths
XQ = R3 + 3                  # q input rows: rho_out + static (43)

# fp16 weight blob column layout: name -> (rows, cols, col_offset)
_BLOB = {}
_off = 0
for _name, _r, _c in [("w1", D, F1), ("w2", F1, F2), ("w3a", F2 + 1, F3),
                      ("r1", F3, R1), ("r2", R1, R2), ("r3", R2, R3),
                      ("q1a", XQ, 128), ("q1b", XQ, Q1 - 128),
                      ("q2a", 128, Q2), ("q2b", Q1 - 128, Q2),
                      ("q3", Q2, Q3), ("statt", D, BC)]:
    _BLOB[_name] = (_r, _c, _off)
    _off += _c
BLOBW = _off

# fp32 bias blob column layout: name -> (rows, col)
_BIAS = {}
for _i, (_name, _r) in enumerate([("b1", F1), ("b2", F2), ("br1", R1),
                                  ("br2", R2), ("br3", R3), ("bq1a", 128),
                                  ("bq1b", Q1 - 128), ("bq2", Q2), ("bq3", Q3),
                                  ("e3_0", 3), ("e3_1", 3), ("e3_2", 3)]):
    _BIAS[_name] = (_r, _i)
BIASW = len(_BIAS)

# per-half L1/L2 copy-engine pattern (ACT-heavy: ACT is the faster copier
# and the DVE also carries the L3 STT folds + pooling reduce)
_COPY_PAT = [0, 1, 0, 1, 0, 1, 0, 0, 1, 0, 1, 0, 1, 0, 1, 0]  # 0=ACT 1=DVE

_compiled = {}


def _build():
    import concourse.bacc as bacc
    import concourse.mybir as mybir
    from concourse import tile

    f32 = mybir.dt.float32
    fp16 = mybir.dt.float16
    Relu = mybir.ActivationFunctionType.Relu
    Ident = mybir.ActivationFunctionType.Identity
    Exp = mybir.ActivationFunctionType.Exp
    Alu = mybir.AluOpType
    AxX = mybir.AxisListType.X

    nc = bacc.Bacc("TRN2", target_bir_lowering=False, debug=False,
                   enable_asserts=False, num_devices=N_CORES)

    xin = nc.dram_tensor("xin", [2, D, HALF], fp16, kind="ExternalInput").ap()
    blob = nc.dram_tensor("blob", [128, BLOBW], fp16, kind="ExternalInput").ap()
    bias = nc.dram_tensor("bias", [128, BIASW], f32, kind="ExternalInput").ap()
    onesr = nc.dram_tensor("onesr", [1, HALF], fp16, kind="ExternalInput").ap()
    out = nc.dram_tensor("out", [BC, 3], f32, kind="ExternalOutput").ap()

    with tile.TileContext(nc) as tc:
        with tc.tile_pool(name="cst", bufs=1) as cst, \
             tc.tile_pool(name="ps", bufs=3, space="PSUM") as ps, \
             tc.tile_pool(name="psw", bufs=1, space="PSUM") as psw, \
             tc.tile_pool(name="pst", bufs=1, space="PSUM") as pst:

            # x half 0 first so the PE can start ASAP (4 column-chunk DMAs)
            XC = HALF // 4
            x_sbs = []
            for h in range(2):
                x_sbs.append(cst.tile([D, HALF], fp16, name=f"x_sb{h}"))

            def dma_x(h):
                for j in range(4):
                    eng = nc.sync if j % 2 == 0 else nc.scalar
                    eng.dma_start(out=x_sbs[h][:, j * XC:(j + 1) * XC],
                                  in_=xin[h, :, j * XC:(j + 1) * XC])

            dma_x(0)
            blob_sb = cst.tile([128, BLOBW], fp16)
            nc.gpsimd.dma_start(out=blob_sb[:, :], in_=blob)
            bias_sb = cst.tile([128, BIASW], f32)
            nc.gpsimd.dma_start(out=bias_sb[:, :], in_=bias)

            def wslice(name):
                r, c, o = _BLOB[name]
                return blob_sb[0:r, o:o + c]

            def bslice(name):
                r, c = _BIAS[name]
                return bias_sb[0:r, c:c + 1]

            w1s, w2s, w3s = wslice("w1"), wslice("w2"), wslice("w3a")
            r1s, r2s, r3s = wslice("r1"), wslice("r2"), wslice("r3")
            q1as, q1bs = wslice("q1a"), wslice("q1b")
            q2as, q2bs = wslice("q2a"), wslice("q2b")
            q3s, statt = wslice("q3"), wslice("statt")
            eye3s = bias_sb[0:3, _BIAS["e3_0"][1]:_BIAS["e3_0"][1] + 3]

            # PE warm-up source (HAM ramp while input DMAs land)
            wtiny = cst.tile([128, 512], fp16)
            nc.vector.memset(wtiny[:, :], 0.0)
            tpre = cst.tile([1, 2], fp16)
            nc.scalar.activation(tpre[:, :], wtiny[0:1, 0:2], Relu)

            # persistent activation planes; h2 gets a ones row for L3's bias
            h1 = cst.tile([F1, HALF], fp16)
            h2 = cst.tile([F2 + 1, HALF], fp16)
            nc.gpsimd.dma_start(out=h2[F2:F2 + 1, :], in_=onesr)
            h3a = cst.tile([F3, HALF // 2], fp16)    # relu'd m in [0,64)
            s_half = cst.tile([F3, HALF // 2], fp16)  # + relu'd m in [64,128)
            t1_half = cst.tile([F3, HALF // 4], fp16)  # gpsimd-folded to 32
            pooled = cst.tile([F3, BC], fp16)
            eye1 = cst.tile([1, 1], f32)
            nc.vector.memset(eye1[:, :], 1.0)
            ones3 = cst.tile([3, 1], f32)
            nc.vector.memset(ones3[:, :], 1.0)

            # per-half tail tiles (static rows preloaded from the blob)
            xqs, hr1s, hr2s, hq1as, hq1bs, hq2s, e_sbs = [], [], [], [], [], [], []
            for h in range(2):
                xqh = cst.tile([XQ, SAMP_HALF], fp16, name=f"xq{h}")
                nc.gpsimd.dma_start(out=xqh[R3:R3 + 3, :],
                                    in_=statt[:, h * SAMP_HALF:(h + 1) * SAMP_HALF])
                xqs.append(xqh)
                hr1s.append(cst.tile([R1, SAMP_HALF], fp16, name=f"hr1{h}"))
                hr2s.append(cst.tile([R2, SAMP_HALF], fp16, name=f"hr2{h}"))
                hq1as.append(cst.tile([128, SAMP_HALF], fp16, name=f"hq1a{h}"))
                hq1bs.append(cst.tile([Q1 - 128, SAMP_HALF], fp16, name=f"hq1b{h}"))
                hq2s.append(cst.tile([Q2, SAMP_HALF], fp16, name=f"hq2{h}"))
                e_sbs.append(cst.tile([3, SAMP_HALF], f32, name=f"e_sb{h}"))

            # warm-up matmuls: >=3.4us of CONTINUOUS PE activity flips the HAM
            # clock gate to 2.4 GHz (9 cold 512-col matmuls ~= 3.8us). The PE
            # then must never sample idle or it re-throttles to 1.2 GHz, so
            # phi tiles interleave filler matmuls (warm_fill) that absorb the
            # PE's slack over the copy engines. Fillers reuse the surrounding
            # layer's stationary so the LDW dedup pass removes their weight
            # reloads entirely.
            pw = psw.tile([128, 512], f32, name="pw")
            for i in range(9):
                nc.tensor.matmul(pw[:, :], wtiny[:, 0:128], wtiny[:, :],
                                 start=True, stop=True)

            def warm_fill(ws, k, n, cols):
                nc.tensor.matmul(pw[0:n, 0:cols], ws, wtiny[0:k, 0:cols],
                                 start=True, stop=True)

            def relu_copy(dst, src, bias_ap, eng):
                if eng == 0:
                    nc.scalar.activation(dst, src, Relu, bias=bias_ap)
                else:
                    nc.vector.tensor_scalar(out=dst, in0=src, scalar1=bias_ap,
                                            scalar2=0.0, op0=Alu.add, op1=Alu.max)

            def mm2(p, rows, ws, src, c0):
                # two 512-col matmuls per 1024-wide PSUM tile (psum-bank limit)
                for cc in range(2):
                    nc.tensor.matmul(p[0:rows, cc * 512:(cc + 1) * 512], ws,
                                     src[:, c0 + cc * 512:c0 + (cc + 1) * 512],
                                     start=True, stop=True)

            FILL = 384

            def phi_l1(h):
                for t in range(NPT):
                    p1 = ps.tile([128, PT], f32, name="p1", tag="hp")
                    mm2(p1, F1, w1s, x_sbs[h], t * PT)
                    warm_fill(w1s, D, F1, FILL)
                    relu_copy(h1[0:F1, t * PT:(t + 1) * PT], p1[0:F1, :],
                              bslice("b1"), _COPY_PAT[t])

            def phi_l2(h):
                for t in range(NPT):
                    p2 = ps.tile([128, PT], f32, name="p2", tag="hp")
                    mm2(p2, F2, w2s, h1, t * PT)
                    warm_fill(w2s, F1, F2, FILL)
                    relu_copy(h2[0:F2, t * PT:(t + 1) * PT], p2[0:F2, :],
                              bslice("b2"), _COPY_PAT[(t + 1) % NPT])

            def phi_l3(h):
                # relu + fold m in [64,128) onto m in [0,64); then per-sample
                # 64-wide sums in 4 chunks so the tail can start early
                for t in range(NPT):
                    p3 = ps.tile([128, PT], f32, name="p3", tag="hp")
                    mm2(p3, F3, w3s, h2, t * PT)
                    warm_fill(w3s, F2 + 1, F3, FILL)
                    p3g = p3[0:F3, :].rearrange("p (g m) -> p g m", m=M)
                    av = h3a[:, t * 512:(t + 1) * 512] \
                        .rearrange("p (g m) -> p g m", m=GS)
                    nc.scalar.activation(av, p3g[:, :, 0:GS], Relu)
                    sv = s_half[:, t * 512:(t + 1) * 512] \
                        .rearrange("p (g m) -> p g m", m=GS)
                    nc.vector.scalar_tensor_tensor(
                        sv, p3g[:, :, GS:M], 0.0, av, op0=Alu.max, op1=Alu.add)
                    if t % 4 == 3:
                        pool_chunk(t // 4, h)

            def pool_chunk(u, h):
                # 64 -> 32 fold on the otherwise-idle GpSimd, then a 32-wide
                # DVE reduce into the pooled plane
                sv = s_half[:, u * 2048:(u + 1) * 2048] \
                    .rearrange("p (s m) -> p s m", m=GS)
                tv = t1_half[:, u * 1024:(u + 1) * 1024] \
                    .rearrange("p (s m) -> p s m", m=GS // 2)
                nc.gpsimd.tensor_tensor(out=tv, in0=sv[:, :, 0:GS // 2],
                                        in1=sv[:, :, GS // 2:GS], op=Alu.add)
                nc.vector.tensor_reduce(
                    out=pooled[:, h * SAMP_HALF + u * 32:
                               h * SAMP_HALF + (u + 1) * 32],
                    in_=tv, axis=AxX, op=Alu.add)

            def tail_stages(h):
                """Yield tail stages so the caller can interleave them with
                the next half's matmul groups (keeps the PE queue busy)."""
                sl = slice(h * SAMP_HALF, (h + 1) * SAMP_HALF)
                xqh, hr1h, hr2h = xqs[h], hr1s[h], hr2s[h]
                hq1ah, hq1bh, hq2h, e_sb = hq1as[h], hq1bs[h], hq2s[h], e_sbs[h]

                def s1():
                    pr1 = pst.tile([R1, SAMP_HALF], f32, name=f"pr1_{h}", tag="tail")
                    nc.tensor.matmul(pr1[:, :], r1s, pooled[:, sl],
                                     start=True, stop=True)
                    nc.scalar.activation(hr1h[:, :], pr1[:, :], Relu,
                                         bias=bslice("br1"))

                def s2():
                    pr2 = pst.tile([R2, SAMP_HALF], f32, name=f"pr2_{h}", tag="tail")
                    nc.tensor.matmul(pr2[:, :], r2s, hr1h[:, :],
                                     start=True, stop=True)
                    nc.scalar.activation(hr2h[:, :], pr2[:, :], Relu,
                                         bias=bslice("br2"))

                def s3():
                    pr3 = pst.tile([R3, SAMP_HALF], f32, name=f"pr3_{h}", tag="tail")
                    nc.tensor.matmul(pr3[:, :], r3s, hr2h[:, :],
                                     start=True, stop=True)
                    nc.scalar.activation(xqh[0:R3, :], pr3[:, :], Ident,
                                         bias=bslice("br3"))

                def s4():
                    pq1a = pst.tile([128, SAMP_HALF], f32, name=f"pq1a_{h}", tag="tail")
                    pq1b = pst.tile([Q1 - 128, SAMP_HALF], f32, name=f"pq1b_{h}",
                                    tag="tail")
                    nc.tensor.matmul(pq1a[:, :], q1as, xqh[:, :],
                                     start=True, stop=True)
                    nc.tensor.matmul(pq1b[:, :], q1bs, xqh[:, :],
                                     start=True, stop=True)
                    nc.scalar.activation(hq1ah[:, :], pq1a[:, :], Relu,
                                         bias=bslice("bq1a"))
                    nc.vector.tensor_scalar(out=hq1bh[:, :], in0=pq1b[:, :],
                                            scalar1=bslice("bq1b"), scalar2=0.0,
                                            op0=Alu.add, op1=Alu.max)

                def s5():
                    pq2 = pst.tile([Q2, SAMP_HALF], f32, name=f"pq2_{h}", tag="tail")
                    nc.tensor.matmul(pq2[:, :], q2as, hq1ah[:, :],
                                     start=True, stop=False)
                    nc.tensor.matmul(pq2[:, :], q2bs, hq1bh[:, :],
                                     start=False, stop=True)
                    nc.scalar.activation(hq2h[:, :], pq2[:, :], Relu,
                                         bias=bslice("bq2"))

                def s6():
                    pq3 = pst.tile([Q3, SAMP_HALF], f32, name=f"pq3_{h}", tag="tail")
                    nc.tensor.matmul(pq3[:, :], q3s, hq2h[:, :],
                                     start=True, stop=True)
                    nc.scalar.activation(e_sb[:, :], pq3[:, :], Exp,
                                         bias=bslice("bq3"))

                def s7():
                    ssum = pst.tile([1, SAMP_HALF], f32, name=f"ssum{h}", tag="tail")
                    nc.tensor.matmul(ssum[:, :], ones3[:, :], e_sb[:, :],
                                     start=True, stop=True)
                    rec = cst.tile([1, SAMP_HALF], f32, name=f"rec{h}")
                    nc.vector.reciprocal(rec[:, :], ssum[:, :])
                    ert = pst.tile([128, 4], f32, name=f"ert{h}", tag="tail")
                    nc.tensor.transpose(ert[:, 0:3], e_sb[:, :], eye3s)
                    nc.tensor.transpose(ert[:, 3:4], rec[:, :], eye1[:, :])
                    rTs = cst.tile([128, 1], f32, name=f"rTs{h}")
                    nc.vector.tensor_copy(rTs[:, :], ert[:, 3:4])
                    o_sb = cst.tile([128, 3], f32, name=f"o_sb{h}")
                    nc.vector.tensor_scalar_mul(o_sb[:, :], ert[:, 0:3], rTs[:, :])
                    nc.sync.dma_start(out=out[h * SAMP_HALF:(h + 1) * SAMP_HALF, :],
                                      in_=o_sb[:, :])

                return [s1, s2, s3, s4, s5, s6, s7]

            with nc.allow_low_precision("fp16 pooled segment sums"):
                phi_l1(0)
                phi_l2(0)
                dma_x(1)
                phi_l3(0)
                phi_l1(1)
                # interleave half-0's serial tail with half-1's L2/L3 groups
                stages = tail_stages(0)
                stages[0]()

                def l2_group(t0, t1):
                    for t in range(t0, t1):
                        p2 = ps.tile([128, PT], f32, name="p2", tag="hp")
                        mm2(p2, F2, w2s, h1, t * PT)
                        warm_fill(w2s, F1, F2, FILL)
                        relu_copy(h2[0:F2, t * PT:(t + 1) * PT], p2[0:F2, :],
                                  bslice("b2"), _COPY_PAT[(t + 1) % NPT])

                def l3_group(t0, t1, h):
                    for t in range(t0, t1):
                        p3 = ps.tile([128, PT], f32, name="p3", tag="hp")
                        mm2(p3, F3, w3s, h2, t * PT)
                        warm_fill(w3s, F2 + 1, F3, FILL)
                        p3g = p3[0:F3, :].rearrange("p (g m) -> p g m", m=M)
                        av = h3a[:, t * 512:(t + 1) * 512] \
                            .rearrange("p (g m) -> p g m", m=GS)
                        nc.scalar.activation(av, p3g[:, :, 0:GS], Relu)
                        sv = s_half[:, t * 512:(t + 1) * 512] \
                            .rearrange("p (g m) -> p g m", m=GS)
                        nc.vector.scalar_tensor_tensor(
                            sv, p3g[:, :, GS:M], 0.0, av, op0=Alu.max, op1=Alu.add)
                        if t % 4 == 3:
                            pool_chunk(t // 4, h)

                l2_group(0, 4)
                stages[1]()
                l2_group(4, 8)
                stages[2]()
                l2_group(8, 12)
                stages[3]()
                l2_group(12, 16)
                stages[4]()
                l3_group(0, 4, 1)
                stages[5]()
                l3_group(4, 8, 1)
                stages[6]()
                l3_group(8, 16, 1)
                for st in tail_stages(1):
                    st()
                    for i in range(3):
                        warm_fill(wtiny[:, 0:128], 128, 128, 512)

    nc.compile()
    _dedup_ldweights(nc)
    return nc


def _prep_inputs(dyn, static, phi_w1, phi_b1, phi_w2, phi_b2, phi_w3, phi_b3,
                 rho_w1, rho_b1, rho_w2, rho_b2, rho_w3, rho_b3,
                 q_w1, q_b1, q_w2, q_b2, q_w3, q_b3):
    """Build the per-core input maps (host-side layout transforms)."""
    fp16 = np.float16

    w3a = np.concatenate([phi_w3, phi_b3[:, None]], axis=1).T.astype(fp16)  # [101,80]
    q1 = q_w1.T.astype(fp16)                 # [43, 200]
    q2 = q_w2.T.astype(fp16)                 # [200, 100]
    parts = dict(
        w1=phi_w1.T.astype(fp16), w2=phi_w2.T.astype(fp16), w3a=w3a,
        r1=rho_w1.T.astype(fp16), r2=rho_w2.T.astype(fp16),
        r3=rho_w3.T.astype(fp16),
        q1a=q1[:, 0:128], q1b=q1[:, 128:],
        q2a=q2[0:128, :], q2b=q2[128:, :], q3=q_w3.T.astype(fp16))

    base_blob = np.zeros((128, BLOBW), dtype=fp16)
    for name, (r, cc, o) in _BLOB.items():
        if name != "statt":
            base_blob[0:r, o:o + cc] = parts[name]

    base_bias = np.zeros((128, BIASW), dtype=np.float32)
    for name, vec in [("b1", phi_b1), ("b2", phi_b2), ("br1", rho_b1),
                      ("br2", rho_b2), ("br3", rho_b3),
                      ("bq1a", q_b1[0:128]), ("bq1b", q_b1[128:]),
                      ("bq2", q_b2), ("bq3", q_b3)]:
        r, c = _BIAS[name]
        base_bias[0:r, c] = vec
    for j in range(3):
        r, c = _BIAS[f"e3_{j}"]
        base_bias[j, c] = 1.0

    in_maps = []
    for c in range(N_CORES):
        blob = base_blob.copy()
        r, cc, o = _BLOB["statt"]
        blob[0:r, o:o + cc] = static[c * BC:(c + 1) * BC].T.astype(fp16)
        xc = dyn[c * BC:(c + 1) * BC].reshape(EC, D).T.astype(fp16)  # [3, EC]
        xin = np.ascontiguousarray(xc.reshape(D, 2, HALF).transpose(1, 0, 2))
        in_maps.append(dict(xin=xin, blob=blob, bias=base_bias,
                            onesr=np.ones((1, HALF), dtype=fp16)))
    return in_maps


def _dedup_ldweights(nc):
    """Drop back-to-back LDWEIGHTS that reload an unchanged stationary: phi
    matmuls reuse one stationary for 32 consecutive 512-col streams, and the
    ~210ns weight reload otherwise serializes with every stream. Only
    wait/update-free reloads are removed, so semaphore ordering is intact."""
    import concourse.mybir as mybir
    dropped = 0
    for b in nc.main_func.blocks:
        last_key = None
        keep = []
        for ins in b.instructions:
            if isinstance(ins, mybir.InstLdweights):
                si = ins.sync_info
                clean = si is None or (len(si.on_wait) == 0
                                       and len(si.on_update) == 0)
                key = (repr(ins.ins[0]), getattr(ins, "tile_position", None),
                       getattr(ins, "perf_mode", None),
                       getattr(ins, "is_transpose", None))
                if clean and key == last_key:
                    dropped += 1
                    continue
                last_key = key
            keep.append(ins)
        b.instructions[:] = keep
    return dropped


def kernel(**inputs):
    import time
    from concourse.bass_utils import run_bass_kernel_spmd

    if "nc" not in _compiled:
        _compiled["nc"] = _build()
    nc = _compiled["nc"]

    in_maps = _prep_inputs(**inputs)
    last_err = None
    for attempt in range(3):
        try:
            res = run_bass_kernel_spmd(nc, in_maps, core_ids=list(range(N_CORES)))
            break
        except Exception as e:          # transient device errors: back off and retry
            last_err = e
            time.sleep(20 * (attempt + 1))
    else:
        raise last_err
    out = np.concatenate([res.results[c]["out"] for c in range(N_CORES)], axis=0)
    return out.astype(np.float32)


# revision 31
# speedup vs baseline: 1.1472x; 1.1472x over previous
"""Trainium2 Bass kernel for the DeepSets-style segment_reduce network.

Network (per sample, B=2048, M=128 elements):
  phi: 3 -> 120 -> 100 -> 80 MLP (all ReLU), applied per element
  pooled = sum over the 128 elements                      [B, 80]
  rho:  80 -> 60 -> 60 -> 40 (ReLU, ReLU, linear)
  q:    concat(rho_out, static) 43 -> 200 -> 100 -> 3, softmax

Mapping: data-parallel over 8 NeuronCores (256 samples each). Activations are
feature-major [features, elements] so each layer is one stationary-weight
matmul sweep of 1024-column fp16 moving operands. Layer biases are folded
into the PSUM->SBUF relu copies (ACT bias operand / DVE tensor_scalar
add+max); only L3 keeps a ones-row (bias via augmented weights) because its
DVE half is a scalar_tensor_tensor that has no bias slot. L3's PSUM read
fuses relu + pairwise element-add (m vs m+64), and the remaining 64-wide
per-sample sum is a chunked DVE tensor_reduce. The rho/q/softmax tail runs
per 128-sample half, its serial chain interleaved between the next half's
matmul groups so the PE never idles long enough to drop out of the warm
2.4 GHz HAM state.
"""

import sys
import numpy as np

sys.path.insert(0, '/opt/trn_rl_repo')

B, M, D = 2048, 128, 3
N_CORES = 8
BC = B // N_CORES            # samples per core (256)
EC = BC * M                  # elements per core (32768)
HALF = EC // 2               # elements per half-sweep (16384)
SAMP_HALF = BC // 2          # samples per half (128)
PT = 1024                    # elements per PSUM tile
NPT = HALF // PT             # psum tiles per half (16)
GS = M // 2                  # pair-summed group width (64)

F1, F2, F3 = 120, 100, 80    # phi widths
R1, R2, R3 = 60, 60, 40      # rho widths
Q1, Q2, Q3 = 200, 100, 3     # q widths
XQ = R3 + 3                  # q input rows: rho_out + static (43)

# fp16 weight blob column layout: name -> (rows, cols, col_offset)
_BLOB = {}
_off = 0
for _name, _r, _c in [("w1", D, F1), ("w2", F1, F2), ("w3a", F2 + 1, F3),
                      ("r1", F3, R1), ("r2", R1, R2), ("r3", R2, R3),
                      ("q1a", XQ, 128), ("q1b", XQ, Q1 - 128),
                      ("q2a", 128, Q2), ("q2b", Q1 - 128, Q2),
                      ("q3", Q2, Q3), ("statt", D, BC)]:
    _BLOB[_name] = (_r, _c, _off)
    _off += _c
BLOBW = _off

# fp32 bias blob column layout: name -> (rows, col)
_BIAS = {}
for _i, (_name, _r) in enumerate([("b1", F1), ("b2", F2), ("br1", R1),
                                  ("br2", R2), ("br3", R3), ("bq1a", 128),
                                  ("bq1b", Q1 - 128), ("bq2", Q2), ("bq3", Q3),
                                  ("e3_0", 3), ("e3_1", 3), ("e3_2", 3)]):
    _BIAS[_name] = (_r, _i)
BIASW = len(_BIAS)

# per-half L1/L2 copy-engine pattern (ACT-heavy: ACT is the faster copier
# and the DVE also carries the L3 STT folds + pooling reduce)
_COPY_PAT = [0, 1, 0, 1, 0, 1, 0, 0, 1, 0, 1, 0, 1, 0, 1, 0]  # 0=ACT 1=DVE

_compiled = {}


def _build():
    import concourse.bacc as bacc
    import concourse.mybir as mybir
    from concourse import tile

    f32 = mybir.dt.float32
    fp16 = mybir.dt.float16
    Relu = mybir.ActivationFunctionType.Relu
    Ident = mybir.ActivationFunctionType.Identity
    Exp = mybir.ActivationFunctionType.Exp
    Alu = mybir.AluOpType
    AxX = mybir.AxisListType.X

    nc = bacc.Bacc("TRN2", target_bir_lowering=False, debug=False,
                   enable_asserts=False, num_devices=N_CORES)

    xin = nc.dram_tensor("xin", [2, D, HALF], fp16, kind="ExternalInput").ap()
    blob = nc.dram_tensor("blob", [128, BLOBW], fp16, kind="ExternalInput").ap()
    bias = nc.dram_tensor("bias", [128, BIASW], f32, kind="ExternalInput").ap()
    onesr = nc.dram_tensor("onesr", [1, HALF], fp16, kind="ExternalInput").ap()
    out = nc.dram_tensor("out", [BC, 3], f32, kind="ExternalOutput").ap()

    with tile.TileContext(nc) as tc:
        with tc.tile_pool(name="cst", bufs=1) as cst, \
             tc.tile_pool(name="ps", bufs=3, space="PSUM") as ps, \
             tc.tile_pool(name="psw", bufs=1, space="PSUM") as psw, \
             tc.tile_pool(name="pst", bufs=1, space="PSUM") as pst:

            # x half 0 first so the PE can start ASAP (4 column-chunk DMAs)
            XC = HALF // 4
            x_sbs = []
            for h in range(2):
                x_sbs.append(cst.tile([D, HALF], fp16, name=f"x_sb{h}"))

            def dma_x(h):
                for j in range(4):
                    eng = nc.sync if j % 2 == 0 else nc.scalar
                    eng.dma_start(out=x_sbs[h][:, j * XC:(j + 1) * XC],
                                  in_=xin[h, :, j * XC:(j + 1) * XC])

            dma_x(0)
            blob_sb = cst.tile([128, BLOBW], fp16)
            nc.gpsimd.dma_start(out=blob_sb[:, :], in_=blob)
            bias_sb = cst.tile([128, BIASW], f32)
            nc.gpsimd.dma_start(out=bias_sb[:, :], in_=bias)

            def wslice(name):
                r, c, o = _BLOB[name]
                return blob_sb[0:r, o:o + c]

            def bslice(name):
                r, c = _BIAS[name]
                return bias_sb[0:r, c:c + 1]

            w1s, w2s, w3s = wslice("w1"), wslice("w2"), wslice("w3a")
            r1s, r2s, r3s = wslice("r1"), wslice("r2"), wslice("r3")
            q1as, q1bs = wslice("q1a"), wslice("q1b")
            q2as, q2bs = wslice("q2a"), wslice("q2b")
            q3s, statt = wslice("q3"), wslice("statt")
            eye3s = bias_sb[0:3, _BIAS["e3_0"][1]:_BIAS["e3_0"][1] + 3]

            # PE warm-up source (HAM ramp while input DMAs land)
            wtiny = cst.tile([128, 512], fp16)
            nc.vector.memset(wtiny[:, :], 0.0)
            tpre = cst.tile([1, 2], fp16)
            nc.scalar.activation(tpre[:, :], wtiny[0:1, 0:2], Relu)

            # persistent activation planes; h2 gets a ones row for L3's bias
            h1 = cst.tile([F1, HALF], fp16)
            h2 = cst.tile([F2 + 1, HALF], fp16)
            nc.gpsimd.dma_start(out=h2[F2:F2 + 1, :], in_=onesr)
            h3a = cst.tile([F3, HALF // 2], fp16)    # relu'd m in [0,64)
            s_half = cst.tile([F3, HALF // 2], fp16)  # + relu'd m in [64,128)
            t1_half = cst.tile([F3, HALF // 4], fp16)  # gpsimd-folded to 32
            pooled = cst.tile([F3, BC], fp16)
            eye1 = cst.tile([1, 1], f32)
            nc.vector.memset(eye1[:, :], 1.0)
            ones3 = cst.tile([3, 1], f32)
            nc.vector.memset(ones3[:, :], 1.0)

            # per-half tail tiles (static rows preloaded from the blob)
            xqs, hr1s, hr2s, hq1as, hq1bs, hq2s, e_sbs = [], [], [], [], [], [], []
            for h in range(2):
                xqh = cst.tile([XQ, SAMP_HALF], fp16, name=f"xq{h}")
                nc.gpsimd.dma_start(out=xqh[R3:R3 + 3, :],
                                    in_=statt[:, h * SAMP_HALF:(h + 1) * SAMP_HALF])
                xqs.append(xqh)
                hr1s.append(cst.tile([R1, SAMP_HALF], fp16, name=f"hr1{h}"))
                hr2s.append(cst.tile([R2, SAMP_HALF], fp16, name=f"hr2{h}"))
                hq1as.append(cst.tile([128, SAMP_HALF], fp16, name=f"hq1a{h}"))
                hq1bs.append(cst.tile([Q1 - 128, SAMP_HALF], fp16, name=f"hq1b{h}"))
                hq2s.append(cst.tile([Q2, SAMP_HALF], fp16, name=f"hq2{h}"))
                e_sbs.append(cst.tile([3, SAMP_HALF], f32, name=f"e_sb{h}"))

            # warm-up matmuls: >=3.4us of CONTINUOUS PE activity flips the HAM
            # clock gate to 2.4 GHz (9 cold 512-col matmuls ~= 3.8us). The PE
            # then must never sample idle or it re-throttles to 1.2 GHz, so
            # phi tiles interleave filler matmuls (warm_fill) that absorb the
            # PE's slack over the copy engines. Fillers reuse the surrounding
            # layer's stationary so the LDW dedup pass removes their weight
            # reloads entirely.
            pw = psw.tile([128, 512], f32, name="pw")
            for i in range(9):
                nc.tensor.matmul(pw[:, :], wtiny[:, 0:128], wtiny[:, :],
                                 start=True, stop=True)

            def warm_fill(ws, k, n, cols):
                nc.tensor.matmul(pw[0:n, 0:cols], ws, wtiny[0:k, 0:cols],
                                 start=True, stop=True)

            def relu_copy(dst, src, bias_ap, eng):
                if eng == 0:
                    nc.scalar.activation(dst, src, Relu, bias=bias_ap)
                else:
                    nc.vector.tensor_scalar(out=dst, in0=src, scalar1=bias_ap,
                                            scalar2=0.0, op0=Alu.add, op1=Alu.max)

            def mm2(p, rows, ws, src, c0):
                # two 512-col matmuls per 1024-wide PSUM tile (psum-bank limit)
                for cc in range(2):
                    nc.tensor.matmul(p[0:rows, cc * 512:(cc + 1) * 512], ws,
                                     src[:, c0 + cc * 512:c0 + (cc + 1) * 512],
                                     start=True, stop=True)

            FILL = 384

            def phi_l1(h):
                for t in range(NPT):
                    p1 = ps.tile([128, PT], f32, name="p1", tag="hp")
                    mm2(p1, F1, w1s, x_sbs[h], t * PT)
                    relu_copy(h1[0:F1, t * PT:(t + 1) * PT], p1[0:F1, :],
                              bslice("b1"), _COPY_PAT[t])

            def phi_l2(h):
                for t in range(NPT):
                    p2 = ps.tile([128, PT], f32, name="p2", tag="hp")
                    mm2(p2, F2, w2s, h1, t * PT)
                    relu_copy(h2[0:F2, t * PT:(t + 1) * PT], p2[0:F2, :],
                              bslice("b2"), _COPY_PAT[(t + 1) % NPT])

            def phi_l3(h):
                # relu + fold m in [64,128) onto m in [0,64); then per-sample
                # 64-wide sums in 4 chunks so the tail can start early
                for t in range(NPT):
                    p3 = ps.tile([128, PT], f32, name="p3", tag="hp")
                    mm2(p3, F3, w3s, h2, t * PT)
                    p3g = p3[0:F3, :].rearrange("p (g m) -> p g m", m=M)
                    av = h3a[:, t * 512:(t + 1) * 512] \
                        .rearrange("p (g m) -> p g m", m=GS)
                    nc.scalar.activation(av, p3g[:, :, 0:GS], Relu)
                    sv = s_half[:, t * 512:(t + 1) * 512] \
                        .rearrange("p (g m) -> p g m", m=GS)
                    nc.vector.scalar_tensor_tensor(
                        sv, p3g[:, :, GS:M], 0.0, av, op0=Alu.max, op1=Alu.add)
                    if t % 4 == 3:
                        pool_chunk(t // 4, h)

            def pool_chunk(u, h):
                # 64 -> 32 fold on the otherwise-idle GpSimd, then a 32-wide
                # DVE reduce into the pooled plane
                sv = s_half[:, u * 2048:(u + 1) * 2048] \
                    .rearrange("p (s m) -> p s m", m=GS)
                tv = t1_half[:, u * 1024:(u + 1) * 1024] \
                    .rearrange("p (s m) -> p s m", m=GS // 2)
                nc.gpsimd.tensor_tensor(out=tv, in0=sv[:, :, 0:GS // 2],
                                        in1=sv[:, :, GS // 2:GS], op=Alu.add)
                nc.vector.tensor_reduce(
                    out=pooled[:, h * SAMP_HALF + u * 32:
                               h * SAMP_HALF + (u + 1) * 32],
                    in_=tv, axis=AxX, op=Alu.add)

            def tail_stages(h):
                """Yield tail stages so the caller can interleave them with
                the next half's matmul groups (keeps the PE queue busy)."""
                sl = slice(h * SAMP_HALF, (h + 1) * SAMP_HALF)
                xqh, hr1h, hr2h = xqs[h], hr1s[h], hr2s[h]
                hq1ah, hq1bh, hq2h, e_sb = hq1as[h], hq1bs[h], hq2s[h], e_sbs[h]

                def s1():
                    pr1 = pst.tile([R1, SAMP_HALF], f32, name=f"pr1_{h}", tag="tail")
                    nc.tensor.matmul(pr1[:, :], r1s, pooled[:, sl],
                                     start=True, stop=True)
                    nc.scalar.activation(hr1h[:, :], pr1[:, :], Relu,
                                         bias=bslice("br1"))

                def s2():
                    pr2 = pst.tile([R2, SAMP_HALF], f32, name=f"pr2_{h}", tag="tail")
                    nc.tensor.matmul(pr2[:, :], r2s, hr1h[:, :],
                                     start=True, stop=True)
                    nc.scalar.activation(hr2h[:, :], pr2[:, :], Relu,
                                         bias=bslice("br2"))

                def s3():
                    pr3 = pst.tile([R3, SAMP_HALF], f32, name=f"pr3_{h}", tag="tail")
                    nc.tensor.matmul(pr3[:, :], r3s, hr2h[:, :],
                                     start=True, stop=True)
                    nc.scalar.activation(xqh[0:R3, :], pr3[:, :], Ident,
                                         bias=bslice("br3"))

                def s4():
                    pq1a = pst.tile([128, SAMP_HALF], f32, name=f"pq1a_{h}", tag="tail")
                    pq1b = pst.tile([Q1 - 128, SAMP_HALF], f32, name=f"pq1b_{h}",
                                    tag="tail")
                    nc.tensor.matmul(pq1a[:, :], q1as, xqh[:, :],
                                     start=True, stop=True)
                    nc.tensor.matmul(pq1b[:, :], q1bs, xqh[:, :],
                                     start=True, stop=True)
                    nc.scalar.activation(hq1ah[:, :], pq1a[:, :], Relu,
                                         bias=bslice("bq1a"))
                    nc.vector.tensor_scalar(out=hq1bh[:, :], in0=pq1b[:, :],
                                            scalar1=bslice("bq1b"), scalar2=0.0,
                                            op0=Alu.add, op1=Alu.max)

                def s5():
                    pq2 = pst.tile([Q2, SAMP_HALF], f32, name=f"pq2_{h}", tag="tail")
                    nc.tensor.matmul(pq2[:, :], q2as, hq1ah[:, :],
                                     start=True, stop=False)
                    nc.tensor.matmul(pq2[:, :], q2bs, hq1bh[:, :],
                                     start=False, stop=True)
                    nc.scalar.activation(hq2h[:, :], pq2[:, :], Relu,
                                         bias=bslice("bq2"))

                def s6():
                    pq3 = pst.tile([Q3, SAMP_HALF], f32, name=f"pq3_{h}", tag="tail")
                    nc.tensor.matmul(pq3[:, :], q3s, hq2h[:, :],
                                     start=True, stop=True)
                    nc.scalar.activation(e_sb[:, :], pq3[:, :], Exp,
                                         bias=bslice("bq3"))

                def s7():
                    ssum = pst.tile([1, SAMP_HALF], f32, name=f"ssum{h}", tag="tail")
                    nc.tensor.matmul(ssum[:, :], ones3[:, :], e_sb[:, :],
                                     start=True, stop=True)
                    rec = cst.tile([1, SAMP_HALF], f32, name=f"rec{h}")
                    nc.vector.reciprocal(rec[:, :], ssum[:, :])
                    ert = pst.tile([128, 4], f32, name=f"ert{h}", tag="tail")
                    nc.tensor.transpose(ert[:, 0:3], e_sb[:, :], eye3s)
                    nc.tensor.transpose(ert[:, 3:4], rec[:, :], eye1[:, :])
                    rTs = cst.tile([128, 1], f32, name=f"rTs{h}")
                    nc.vector.tensor_copy(rTs[:, :], ert[:, 3:4])
                    o_sb = cst.tile([128, 3], f32, name=f"o_sb{h}")
                    nc.vector.tensor_scalar_mul(o_sb[:, :], ert[:, 0:3], rTs[:, :])
                    nc.sync.dma_start(out=out[h * SAMP_HALF:(h + 1) * SAMP_HALF, :],
                                      in_=o_sb[:, :])

                return [s1, s2, s3, s4, s5, s6, s7]

            with nc.allow_low_precision("fp16 pooled segment sums"):
                phi_l1(0)
                phi_l2(0)
                dma_x(1)
                phi_l3(0)
                phi_l1(1)
                # interleave half-0's serial tail with half-1's L2/L3 groups
                stages = tail_stages(0)
                stages[0]()

                def l2_group(t0, t1):
                    for t in range(t0, t1):
                        p2 = ps.tile([128, PT], f32, name="p2", tag="hp")
                        mm2(p2, F2, w2s, h1, t * PT)
                        relu_copy(h2[0:F2, t * PT:(t + 1) * PT], p2[0:F2, :],
                                  bslice("b2"), _COPY_PAT[(t + 1) % NPT])

                def l3_group(t0, t1, h):
                    for t in range(t0, t1):
                        p3 = ps.tile([128, PT], f32, name="p3", tag="hp")
                        mm2(p3, F3, w3s, h2, t * PT)
                        p3g = p3[0:F3, :].rearrange("p (g m) -> p g m", m=M)
                        av = h3a[:, t * 512:(t + 1) * 512] \
                            .rearrange("p (g m) -> p g m", m=GS)
                        nc.scalar.activation(av, p3g[:, :, 0:GS], Relu)
                        sv = s_half[:, t * 512:(t + 1) * 512] \
                            .rearrange("p (g m) -> p g m", m=GS)
                        nc.vector.scalar_tensor_tensor(
                            sv, p3g[:, :, GS:M], 0.0, av, op0=Alu.max, op1=Alu.add)
                        if t % 4 == 3:
                            pool_chunk(t // 4, h)

                l2_group(0, 4)
                stages[1]()
                l2_group(4, 8)
                stages[2]()
                l2_group(8, 12)
                stages[3]()
                l2_group(12, 16)
                stages[4]()
                l3_group(0, 4, 1)
                stages[5]()
                l3_group(4, 8, 1)
                stages[6]()
                l3_group(8, 16, 1)
                for st in tail_stages(1):
                    st()

    nc.compile()
    _dedup_ldweights(nc)
    return nc


def _prep_inputs(dyn, static, phi_w1, phi_b1, phi_w2, phi_b2, phi_w3, phi_b3,
                 rho_w1, rho_b1, rho_w2, rho_b2, rho_w3, rho_b3,
                 q_w1, q_b1, q_w2, q_b2, q_w3, q_b3):
    """Build the per-core input maps (host-side layout transforms)."""
    fp16 = np.float16

    w3a = np.concatenate([phi_w3, phi_b3[:, None]], axis=1).T.astype(fp16)  # [101,80]
    q1 = q_w1.T.astype(fp16)                 # [43, 200]
    q2 = q_w2.T.astype(fp16)                 # [200, 100]
    parts = dict(
        w1=phi_w1.T.astype(fp16), w2=phi_w2.T.astype(fp16), w3a=w3a,
        r1=rho_w1.T.astype(fp16), r2=rho_w2.T.astype(fp16),
        r3=rho_w3.T.astype(fp16),
        q1a=q1[:, 0:128], q1b=q1[:, 128:],
        q2a=q2[0:128, :], q2b=q2[128:, :], q3=q_w3.T.astype(fp16))

    base_blob = np.zeros((128, BLOBW), dtype=fp16)
    for name, (r, cc, o) in _BLOB.items():
        if name != "statt":
            base_blob[0:r, o:o + cc] = parts[name]

    base_bias = np.zeros((128, BIASW), dtype=np.float32)
    for name, vec in [("b1", phi_b1), ("b2", phi_b2), ("br1", rho_b1),
                      ("br2", rho_b2), ("br3", rho_b3),
                      ("bq1a", q_b1[0:128]), ("bq1b", q_b1[128:]),
                      ("bq2", q_b2), ("bq3", q_b3)]:
        r, c = _BIAS[name]
        base_bias[0:r, c] = vec
    for j in range(3):
        r, c = _BIAS[f"e3_{j}"]
        base_bias[j, c] = 1.0

    in_maps = []
    for c in range(N_CORES):
        blob = base_blob.copy()
        r, cc, o = _BLOB["statt"]
        blob[0:r, o:o + cc] = static[c * BC:(c + 1) * BC].T.astype(fp16)
        xc = dyn[c * BC:(c + 1) * BC].reshape(EC, D).T.astype(fp16)  # [3, EC]
        xin = np.ascontiguousarray(xc.reshape(D, 2, HALF).transpose(1, 0, 2))
        in_maps.append(dict(xin=xin, blob=blob, bias=base_bias,
                            onesr=np.ones((1, HALF), dtype=fp16)))
    return in_maps


def _dedup_ldweights(nc):
    """Drop back-to-back LDWEIGHTS that reload an unchanged stationary: phi
    matmuls reuse one stationary for 32 consecutive 512-col streams, and the
    ~210ns weight reload otherwise serializes with every stream. Only
    wait/update-free reloads are removed, so semaphore ordering is intact."""
    import concourse.mybir as mybir
    dropped = 0
    for b in nc.main_func.blocks:
        last_key = None
        keep = []
        for ins in b.instructions:
            if isinstance(ins, mybir.InstLdweights):
                si = ins.sync_info
                clean = si is None or (len(si.on_wait) == 0
                                       and len(si.on_update) == 0)
                key = (repr(ins.ins[0]), getattr(ins, "tile_position", None),
                       getattr(ins, "perf_mode", None),
                       getattr(ins, "is_transpose", None))
                if clean and key == last_key:
                    dropped += 1
                    continue
                last_key = key
            keep.append(ins)
        b.instructions[:] = keep
    return dropped


def kernel(**inputs):
    import time
    from concourse.bass_utils import run_bass_kernel_spmd

    if "nc" not in _compiled:
        _compiled["nc"] = _build()
    nc = _compiled["nc"]

    in_maps = _prep_inputs(**inputs)
    last_err = None
    for attempt in range(3):
        try:
            res = run_bass_kernel_spmd(nc, in_maps, core_ids=list(range(N_CORES)))
            break
        except Exception as e:          # transient device errors: back off and retry
            last_err = e
            time.sleep(20 * (attempt + 1))
    else:
        raise last_err
    out = np.concatenate([res.results[c]["out"] for c in range(N_CORES)], axis=0)
    return out.astype(np.float32)
